# revision 18
# baseline (speedup 1.0000x reference)
"""Trainium2 Bass kernel for nn_MultiHeadAttn (B=2, S=2048, D=1024, H=16,
ADIM=64, rel-pos bias vocab 33).

Sharding: batch x head-group over 8 cores. Core c handles batch b=c//4 and
heads [4*(c%4), 4*(c%4)+4). Each core computes q/k/v projections for its 256
model dims, attention for its 4 heads, and a partial output projection; the
host sums the 4 bf16 partials per batch in fp32.

Rel-pos bias: scoresT[s,t] = (q_t/8).k_s with k VARIANTS so the far field is
free (kLo = k + pemb[32] for s-t >= 256, kHi = k + pemb[0] for t-s >= 256);
the up-to-3 diagonal-crossing 128-wide t-subtiles per s-tile use plain k and
get their bias MULTIPLICATIVELY after exp via one fused DVE multiply with a
host-precomputed band = exp((q/8).pemb[clamp(s-t+16,0,32)]).

Softmax runs without max subtraction (logits bounded ~|4|); the denominator
comes from a ones column appended to v.

AV runs with v STATIONARY (lhsT = v_aug[st] [128s, 65]) streaming expT,
accumulating ctxT_aug [65, 1024] in PSUM per (head, t-half): row 64 is the
softmax denominator, rows 0..63 are ctx^T - exactly the layout the output
projection consumes as lhsT, so no PE transposes. AV issues two groups
behind its scores group (the scores->exp->band chain is ~2.5us deep).
Normalization: denominator row -> sbuf copy -> reciprocal_approx_fast ->
gpsimd partition_broadcast -> one DVE multiply; double-buffered ctx psum
tiles keep it off the critical path.

Phase order: [k/q/v projections] -> heads 0..3 -> out-projection (inside
the attention pool scope, drawing PSUM from the scores ring, so there is no
scope-exit barrier and it overlaps the final normalize).
"""
import numpy as np
import ml_dtypes

import concourse.bacc as bacc
import concourse.mybir as mybir
import concourse.tile as tile
from concourse.bass_utils import run_bass_kernel_spmd

B, S, D = 2, 2048, 1024
H, ADIM, K_REL, NJ = 16, 64, 16, 33
HPC = 4            # heads per core
DHC = HPC * ADIM   # 256 model dims per core
P = 128
NST = S // P       # 16 s-tiles
NKC = D // P       # 8 contraction chunks for projections
TH = S // 2        # 1024 t-cols per half
BF16 = mybir.dt.bfloat16
FP8 = mybir.dt.float8e4
FP32 = mybir.dt.float32

_COMPILED = None


def build_nc():
    nc = bacc.Bacc(None, target_bir_lowering=False)
    with tile.TileContext(nc) as tc:
        from contextlib import ExitStack
        x_d = {nm: nc.dram_tensor(f"x{nm}", [P, NKC * S], BF16,
                                  kind="ExternalInput") for nm in "qkv"}
        w_d = {nm: nc.dram_tensor(f"w{nm}", [P, NKC * DHC], BF16,
                                  kind="ExternalInput") for nm in "qkv"}
        wo_d = nc.dram_tensor("wo", [P, 2 * D], BF16, kind="ExternalInput")
        pemb_d = nc.dram_tensor("pemb", [P, 2], FP32, kind="ExternalInput")
        band_d = nc.dram_tensor("band", [HPC, P, NST * 3 * P], BF16,
                                kind="ExternalInput")
        out_d = nc.dram_tensor("out", [S, D], BF16, kind="ExternalOutput")

        with ExitStack() as stack:
            const = stack.enter_context(tc.tile_pool(name="const", bufs=1))
            pemb_sb = const.tile([P, 2], FP32)
            nc.sync.dma_start(out=pemb_sb[:], in_=pemb_d[:])

            persist = stack.enter_context(tc.tile_pool(name="persist", bufs=1))
            qT_sb = [persist.tile([P, S], BF16, name=f"qT{i}") for i in range(2)]
            kT_sb = [persist.tile([P, S], BF16, name=f"kT{i}") for i in range(2)]
            kLo_sb = [persist.tile([P, S], BF16, name=f"kLo{i}") for i in range(2)]
            kHi_sb = [persist.tile([P, S], BF16, name=f"kHi{i}") for i in range(2)]
            v_sb = [persist.tile([P, HPC * 65], BF16, name=f"v{st}")
                    for st in range(NST)]
            ctxT_sb = [persist.tile([P, S], BF16, name=f"ctxT{i}")
                       for i in range(2)]
            wo_sb = persist.tile([P, 2 * D], BF16, name="wo")

            pqk = stack.enter_context(tc.tile_pool(name="xqk", bufs=1))
            x_sb = {nm: pqk.tile([P, NKC * S], BF16, name=f"x{nm}")
                    for nm in "qk"}
            w_sb = {nm: pqk.tile([P, NKC * DHC], BF16, name=f"w{nm}")
                    for nm in "qk"}

            def proj_qk(nm, mt, psum_pool, dst):
                ps4 = [psum_pool.tile([P, 512], FP32, name="proj")
                       for _ in range(4)]
                for kc in range(NKC):
                    for nb in range(4):
                        nc.tensor.matmul(
                            ps4[nb][:],
                            lhsT=w_sb[nm][:, kc * DHC + mt * P:
                                          kc * DHC + mt * P + P],
                            rhs=x_sb[nm][:, kc * S + nb * 512:
                                         kc * S + nb * 512 + 512],
                            start=(kc == 0), stop=(kc == NKC - 1))
                for nb in range(4):
                    nc.vector.tensor_copy(
                        dst[mt][:, nb * 512:nb * 512 + 512], ps4[nb][:])

            def klohi(mt):
                nc.vector.tensor_scalar_add(
                    kLo_sb[mt][:], kT_sb[mt][:], pemb_sb[:, 1:2])
                nc.vector.tensor_scalar_add(
                    kHi_sb[mt][:], kT_sb[mt][:], pemb_sb[:, 0:1])

            # ---- P1a: k/q mt0 + v ----
            with ExitStack() as p1a:
                pv = p1a.enter_context(tc.tile_pool(name="xv", bufs=1))
                ppsum = p1a.enter_context(
                    tc.tile_pool(name="ppsum", bufs=6, space="PSUM"))
                vpsum = p1a.enter_context(
                    tc.tile_pool(name="vpsum", bufs=2, space="PSUM"))
                xv_sb = pv.tile([P, NKC * S], BF16, name="xv")
                wv_sb = pv.tile([P, NKC * DHC], BF16, name="wv")
                # DMA issue order == PE consumption order: k, q, v;
                # first two band tiles slip in so head 0/1 never wait
                for nm in "kq":
                    nc.sync.dma_start(out=w_sb[nm][:], in_=w_d[nm][:])
                    hw = NKC * S // 4
                    for ch in range(4):
                        nc.sync.dma_start(
                            out=x_sb[nm][:, ch * hw:(ch + 1) * hw],
                            in_=x_d[nm][:, ch * hw:(ch + 1) * hw])
                nc.sync.dma_start(out=wv_sb[:], in_=w_d["v"][:])
                hw = NKC * S // 4
                for ch in range(4):
                    nc.sync.dma_start(
                        out=xv_sb[:, ch * hw:(ch + 1) * hw],
                        in_=x_d["v"][:, ch * hw:(ch + 1) * hw])
                nc.sync.dma_start(out=wo_sb[:], in_=wo_d[:])

                proj_qk("k", 0, ppsum, kT_sb)
                klohi(0)
                proj_qk("q", 0, ppsum, qT_sb)
                proj_qk("k", 1, ppsum, kT_sb)
                klohi(1)
                proj_qk("q", 1, ppsum, qT_sb)
                for st in range(NST):
                    ps = vpsum.tile([P, DHC], FP32, name="projv")
                    for kc in range(NKC):
                        nc.tensor.matmul(
                            ps[:],
                            lhsT=xv_sb[:, kc * S + st * P:kc * S + st * P + P],
                            rhs=wv_sb[:, kc * DHC:(kc + 1) * DHC],
                            start=(kc == 0), stop=(kc == NKC - 1))
                    nc.vector.memset(v_sb[st][:], 1.0)
                    for h in range(HPC):
                        nc.vector.tensor_copy(
                            v_sb[st][:, 65 * h:65 * h + ADIM],
                            ps[:, ADIM * h:ADIM * h + ADIM])

            # ---- attention for a pair of heads (own pool scope) ----
            def run_heads(p3, heads, tail=None):
                spsum = p3.enter_context(
                    tc.tile_pool(name="spsum", bufs=2, space="PSUM"))
                cpsum = p3.enter_context(
                    tc.tile_pool(name="cpsum", bufs=2, space="PSUM"))
                epool = p3.enter_context(tc.tile_pool(name="expT", bufs=4))
                bpool = p3.enter_context(tc.tile_pool(name="band", bufs=3))
                rpool = p3.enter_context(tc.tile_pool(name="recip", bufs=2))
                gpool = p3.enter_context(tc.tile_pool(name="recB", bufs=2))
                ksrc = (kT_sb, kLo_sb, kHi_sb)
                band_tiles, ctx_tiles = {}, {}

                def issue_scores(h, th, st):
                    mt, po = h // 2, ADIM * (h % 2)
                    s0 = st * P
                    if th == 0 and st == 0:
                        bt = bpool.tile([P, NST * 3 * P], BF16, name="band")
                        nc.sync.dma_start(out=bt[:], in_=band_d[h])
                        band_tiles[h] = bt
                    band_sb = band_tiles[h]
                    expT = epool.tile([P, TH], BF16, name="expT")
                    ps = spsum.tile([P, TH], FP32, name="scores")
                    runs = []
                    for tt in range(8 * th, 8 * th + 8):
                        dd = st - tt
                        kv = 1 if dd >= 2 else (2 if dd <= -2 else 0)
                        if runs and runs[-1][2] == kv and (tt % 4) != 0:
                            runs[-1][1] = tt + 1
                        else:
                            runs.append([tt, tt + 1, kv])
                    for ta, tb, kv in runs:
                        co = (ta - 8 * th) * P
                        nc.tensor.matmul(
                            ps[:, co:co + (tb - ta) * P],
                            lhsT=ksrc[kv][mt][po:po + ADIM, s0:s0 + P],
                            rhs=qT_sb[mt][po:po + ADIM, ta * P:tb * P],
                            start=True, stop=True)
                    nc.scalar.activation(
                        expT[:], ps[:], mybir.ActivationFunctionType.Exp)
                    ta = max(st - 1, 8 * th)
                    tb = min(st + 2, 8 * th + 8)
                    if ta < tb:
                        bo = (st * 3 + (ta - st + 1)) * P
                        co = (ta - 8 * th) * P
                        w_band = (tb - ta) * P
                        nc.vector.tensor_mul(
                            expT[:, co:co + w_band],
                            expT[:, co:co + w_band],
                            band_sb[:, bo:bo + w_band])
                    return expT

                def issue_av(h, th, st, expT):
                    if st == 0:
                        ctx_tiles[(h, th)] = cpsum.tile([P, TH], FP32,
                                                        name="ctx")
                    ctx_ps = ctx_tiles[(h, th)]
                    for nb in range(2):
                        nc.tensor.matmul(
                            ctx_ps[0:65, nb * 512:nb * 512 + 512],
                            lhsT=v_sb[st][:, 65 * h:65 * h + 65],
                            rhs=expT[:, nb * 512:nb * 512 + 512],
                            start=(st == 0), stop=(st == NST - 1))
                    if st == NST - 1:
                        mt, po = h // 2, ADIM * (h % 2)
                        den = rpool.tile([1, TH], FP32, name="den")
                        rec = rpool.tile([1, TH], FP32, name="rec")
                        recB = gpool.tile([ADIM, TH], FP32, name="recB")
                        # psum row -> sbuf first: approx recip's bit trick
                        # needs IEEE fp32, not the e10m23 psum format
                        nc.vector.tensor_copy(den[:], ctx_ps[ADIM:ADIM + 1, :])
                        nc.vector.reciprocal_approx_fast(rec[:], den[:])
                        nc.gpsimd.partition_broadcast(recB[:], rec[0:1, :])
                        nc.vector.tensor_mul(
                            ctxT_sb[mt][po:po + ADIM, th * TH:th * TH + TH],
                            ctx_ps[0:ADIM, :], recB[:])

                from collections import deque
                pend = deque()
                for g in [(h, th, st) for h in heads
                          for th in range(2) for st in range(NST)]:
                    expT = issue_scores(*g)
                    pend.append((g, expT))
                    if len(pend) > 2:   # AV runs two groups behind scores
                        pg, pe = pend.popleft()
                        issue_av(*pg, pe)
                while pend:
                    pg, pe = pend.popleft()
                    issue_av(*pg, pe)
                if tail is not None:
                    for tt in range(NST):
                        tail((spsum, cpsum), tt)

            # ---- P4: out projection, inside the heads {2,3} scope,
            # drawing psum from the scores ring (no scope-exit barrier) ----
            ostage = stack.enter_context(tc.tile_pool(name="ostage", bufs=3))

            def out_proj(pools, tt):
                spsum, cpsum = pools
                st_t = ostage.tile([P, D], BF16, name="ost")
                for nb in range(2):
                    if (2 * tt + nb) % 2 == 0:
                        ps = spsum.tile([P, TH], FP32, name="scores")
                    else:
                        ps = cpsum.tile([P, TH], FP32, name="ctx")
                    for cc in range(2):
                        nc.tensor.matmul(
                            ps[:, 0:512],
                            lhsT=ctxT_sb[cc][:, tt * P:tt * P + P],
                            rhs=wo_sb[:, cc * D + nb * 512:
                                      cc * D + nb * 512 + 512],
                            start=(cc == 0), stop=(cc == 1))
                    if nb == 0:
                        nc.scalar.activation(
                            st_t[:, 0:512], ps[:, 0:512],
                            mybir.ActivationFunctionType.Copy)
                    else:
                        nc.vector.tensor_copy(
                            st_t[:, 512:1024], ps[:, 0:512])
                    nc.sync.dma_start(
                        out=out_d[tt * P:tt * P + P,
                                  nb * 512:nb * 512 + 512],
                        in_=st_t[:, nb * 512:nb * 512 + 512])

            with ExitStack() as p3:
                run_heads(p3, (0, 1, 2, 3), tail=out_proj)
    nc.compile()
    return nc


def _bf16(x):
    return np.ascontiguousarray(np.asarray(x, np.float32)).astype(
        ml_dtypes.bfloat16)


def _fp8(x):
    x = np.clip(np.ascontiguousarray(np.asarray(x, np.float32)), -240, 240)
    return x.astype(ml_dtypes.float8_e4m3fn)


def _swiz(xT):
    """[D, S]-like -> SBUF layout [128, (D/128)*S] (chunk kc at cols kc*S)."""
    d0, s0 = xT.shape
    return np.ascontiguousarray(
        xT.reshape(d0 // P, P, s0).transpose(1, 0, 2).reshape(P, -1))


def _host_inputs(iQ, iK, iV, Wq, Wk, Wv, Wo, rel_pemb):
    iQ, iK, iV = (np.asarray(a, np.float32) for a in (iQ, iK, iV))
    Wq, Wk, Wv, Wo = (np.asarray(a, np.float32) for a in (Wq, Wk, Wv, Wo))
    rel_pemb = np.asarray(rel_pemb, np.float32)
    iQ = iQ * 0.125  # fold the 1/sqrt(ADIM) scale into the q input
    pembT = rel_pemb.T
    pemb = np.stack([np.tile(rel_pemb[0], 2), np.tile(rel_pemb[32], 2)],
                    axis=1).astype(np.float32)  # [128, 2]: col0=hi, col1=lo

    sl = np.arange(P)[:, None]
    tl = np.arange(P)[None, :]
    idx_d = {d: np.clip(d + sl - tl + K_REL, 0, NJ - 1) for d in (128, 0, -128)}
    slot_d = (128, 0, -128)

    in_maps = []
    for c in range(8):
        b, g = c // 4, c % 4
        cols = slice(DHC * g, DHC * g + DHC)
        Qg = iQ[b] @ Wq[:, cols]  # already includes the 1/8 scale
        band = np.zeros((HPC, NST, 3, P, P), np.float32)
        for h in range(HPC):
            ph = Qg[:, ADIM * h:ADIM * h + ADIM] @ pembT
            for st in range(NST):
                for slot, d in enumerate(slot_d):
                    tt = st - 1 + slot
                    if not 0 <= tt < NST:
                        continue
                    pb = ph[tt * P:tt * P + P]
                    band[h, st, slot] = pb[tl, idx_d[d]]
        band = np.exp(band)
        # -> [HPC, 128(sl), NST*3*128(tl-groups)]
        band = np.ascontiguousarray(band.transpose(0, 3, 1, 2, 4)
                                    .reshape(HPC, P, NST * 3 * P))
        in_maps.append({
            "xq": _bf16(_swiz(iQ[b].T)), "xk": _bf16(_swiz(iK[b].T)),
            "xv": _bf16(_swiz(iV[b].T)),
            "wq": _bf16(_swiz(Wq[:, cols])), "wk": _bf16(_swiz(Wk[:, cols])),
            "wv": _bf16(_swiz(Wv[:, cols])), "wo": _bf16(_swiz(Wo[cols, :])),
            "pemb": pemb, "band": _bf16(band),
        })
    return in_maps


def kernel(iQ, iK, iV, Wq, Wk, Wv, Wo, rel_pemb, _trace=False):
    global _COMPILED
    if _COMPILED is None:
        _COMPILED = build_nc()
    nc = _COMPILED
    in_maps = _host_inputs(iQ, iK, iV, Wq, Wk, Wv, Wo, rel_pemb)
    res = run_bass_kernel_spmd(nc, in_maps, list(range(8)), trace=_trace)
    parts = [res.results[c]["out"].astype(np.float32) for c in range(8)]
    out = np.stack([parts[0] + parts[1] + parts[2] + parts[3],
                    parts[4] + parts[5] + parts[6] + parts[7]])
    if _trace:
        return out, res
    return out


# revision 19
# speedup vs baseline: 1.1238x; 1.1238x over previous
"""Trainium2 Bass kernel for nn_MultiHeadAttn (B=2, S=2048, D=1024, H=16,
ADIM=64, rel-pos bias vocab 33).

Sharding: batch x head-group over 8 cores. Core c handles batch b=c//4 and
heads [4*(c%4), 4*(c%4)+4). Each core computes q/k/v projections for its 256
model dims, attention for its 4 heads, and a partial output projection; the
host sums the 4 bf16 partials per batch in fp32.

Rel-pos bias: scoresT[s,t] = (q_t/8).k_s with k VARIANTS so the far field is
free (kLo = k + pemb[32] for s-t >= 256, kHi = k + pemb[0] for t-s >= 256);
the up-to-3 diagonal-crossing 128-wide t-subtiles per s-tile use plain k and
get their bias MULTIPLICATIVELY after exp via one fused DVE multiply with a
host-precomputed band = exp((q/8).pemb[clamp(s-t+16,0,32)]).

Softmax runs without max subtraction (logits bounded ~|4|); the denominator
comes from a ones column appended to v.

AV runs with v STATIONARY (lhsT = v_aug[st] [128s, 65]) streaming expT,
accumulating ctxT_aug [65, 1024] in PSUM per (head, t-half): row 64 is the
softmax denominator, rows 0..63 are ctx^T - exactly the layout the output
projection consumes as lhsT, so no PE transposes. AV issues two groups
behind its scores group (the scores->exp->band chain is ~2.5us deep).
Normalization: denominator row -> sbuf copy -> reciprocal_approx_fast ->
gpsimd partition_broadcast -> one DVE multiply; double-buffered ctx psum
tiles keep it off the critical path.

Phase order: [k/q/v projections] -> heads 0..3 -> out-projection (inside
the attention pool scope, drawing PSUM from the scores ring, so there is no
scope-exit barrier and it overlaps the final normalize).
"""
import numpy as np
import ml_dtypes

import concourse.bacc as bacc
import concourse.mybir as mybir
import concourse.tile as tile
from concourse.bass_utils import run_bass_kernel_spmd

B, S, D = 2, 2048, 1024
H, ADIM, K_REL, NJ = 16, 64, 16, 33
HPC = 4            # heads per core
DHC = HPC * ADIM   # 256 model dims per core
P = 128
NST = S // P       # 16 s-tiles
NKC = D // P       # 8 contraction chunks for projections
TH = S // 2        # 1024 t-cols per half
BF16 = mybir.dt.bfloat16
FP8 = mybir.dt.float8e4
FP32 = mybir.dt.float32

_COMPILED = None


def build_nc():
    nc = bacc.Bacc(None, target_bir_lowering=False)
    with tile.TileContext(nc) as tc:
        from contextlib import ExitStack
        x_d = {nm: nc.dram_tensor(f"x{nm}", [P, NKC * S], BF16,
                                  kind="ExternalInput") for nm in "qkv"}
        w_d = {nm: nc.dram_tensor(f"w{nm}", [P, NKC * DHC], BF16,
                                  kind="ExternalInput") for nm in "qkv"}
        wo_d = nc.dram_tensor("wo", [P, 2 * D], BF16, kind="ExternalInput")
        pemb_d = nc.dram_tensor("pemb", [P, 2], FP32, kind="ExternalInput")
        band_d = nc.dram_tensor("band", [HPC, P, NST * 3 * P], BF16,
                                kind="ExternalInput")
        out_d = nc.dram_tensor("out", [S, D], BF16, kind="ExternalOutput")

        with ExitStack() as stack:
            const = stack.enter_context(tc.tile_pool(name="const", bufs=1))
            pemb_sb = const.tile([P, 2], FP32)
            nc.sync.dma_start(out=pemb_sb[:], in_=pemb_d[:])

            persist = stack.enter_context(tc.tile_pool(name="persist", bufs=1))
            qT_sb = [persist.tile([P, S], BF16, name=f"qT{i}") for i in range(2)]
            kT_sb = [persist.tile([P, S], BF16, name=f"kT{i}") for i in range(2)]
            kLo_sb = [persist.tile([P, S], BF16, name=f"kLo{i}") for i in range(2)]
            kHi_sb = [persist.tile([P, S], BF16, name=f"kHi{i}") for i in range(2)]
            v_sb = [persist.tile([P, HPC * 65], BF16, name=f"v{st}")
                    for st in range(NST)]
            ctxT_sb = [persist.tile([P, S], BF16, name=f"ctxT{i}")
                       for i in range(2)]
            wo_sb = persist.tile([P, 2 * D], BF16, name="wo")

            pqk = stack.enter_context(tc.tile_pool(name="xqk", bufs=1))
            x_sb = {nm: pqk.tile([P, NKC * S], BF16, name=f"x{nm}")
                    for nm in "qk"}
            w_sb = {nm: pqk.tile([P, NKC * DHC], BF16, name=f"w{nm}")
                    for nm in "qk"}

            def proj_qk(nm, mt, psum_pool, dst):
                ps4 = [psum_pool.tile([P, 512], FP32, name="proj")
                       for _ in range(4)]
                for kc in range(NKC):
                    for nb in range(4):
                        nc.tensor.matmul(
                            ps4[nb][:],
                            lhsT=w_sb[nm][:, kc * DHC + mt * P:
                                          kc * DHC + mt * P + P],
                            rhs=x_sb[nm][:, kc * S + nb * 512:
                                         kc * S + nb * 512 + 512],
                            start=(kc == 0), stop=(kc == NKC - 1))
                for nb in range(4):
                    nc.vector.tensor_copy(
                        dst[mt][:, nb * 512:nb * 512 + 512], ps4[nb][:])

            def klohi(mt):
                nc.vector.tensor_scalar_add(
                    kLo_sb[mt][:], kT_sb[mt][:], pemb_sb[:, 1:2])
                nc.vector.tensor_scalar_add(
                    kHi_sb[mt][:], kT_sb[mt][:], pemb_sb[:, 0:1])

            # ---- P1a: k/q mt0 + v ----
            with ExitStack() as p1a:
                pv = p1a.enter_context(tc.tile_pool(name="xv", bufs=1))
                ppsum = p1a.enter_context(
                    tc.tile_pool(name="ppsum", bufs=6, space="PSUM"))
                vpsum = p1a.enter_context(
                    tc.tile_pool(name="vpsum", bufs=2, space="PSUM"))
                xv_sb = pv.tile([P, NKC * S], BF16, name="xv")
                wv_sb = pv.tile([P, NKC * DHC], BF16, name="wv")
                # DMA issue order == PE consumption order: k, q, v;
                # first two band tiles slip in so head 0/1 never wait
                for nm in "kq":
                    nc.sync.dma_start(out=w_sb[nm][:], in_=w_d[nm][:])
                    hw = NKC * S // 4
                    for ch in range(4):
                        nc.sync.dma_start(
                            out=x_sb[nm][:, ch * hw:(ch + 1) * hw],
                            in_=x_d[nm][:, ch * hw:(ch + 1) * hw])
                nc.sync.dma_start(out=wv_sb[:], in_=w_d["v"][:])
                hw = NKC * S // 4
                for ch in range(4):
                    nc.sync.dma_start(
                        out=xv_sb[:, ch * hw:(ch + 1) * hw],
                        in_=x_d["v"][:, ch * hw:(ch + 1) * hw])
                nc.sync.dma_start(out=wo_sb[:], in_=wo_d[:])

                # keep the PE busy during the DMA lead-in so the HAM
                # clock gate stays at 8/8 into the projection phase
                warm = pv.tile([P, 512], BF16, name="warm")
                nc.vector.memset(warm[:], 0.0)
                wps = ppsum.tile([P, 512], FP32, name="proj")
                for _ in range(48):
                    nc.tensor.matmul(wps[:], lhsT=warm[:, 0:P],
                                     rhs=warm[:], start=True, stop=True)

                proj_qk("k", 0, ppsum, kT_sb)
                klohi(0)
                proj_qk("q", 0, ppsum, qT_sb)
                proj_qk("k", 1, ppsum, kT_sb)
                klohi(1)
                proj_qk("q", 1, ppsum, qT_sb)
                for st in range(NST):
                    ps = vpsum.tile([P, DHC], FP32, name="projv")
                    ch, so = st // 4, (st % 4) * P
                    for kc in range(NKC):
                        nc.tensor.matmul(
                            ps[:],
                            lhsT=xv_sb[:, ch * (NKC * 512) + kc * 512 + so:
                                       ch * (NKC * 512) + kc * 512 + so + P],
                            rhs=wv_sb[:, kc * DHC:(kc + 1) * DHC],
                            start=(kc == 0), stop=(kc == NKC - 1))
                    nc.vector.memset(v_sb[st][:], 1.0)
                    for h in range(HPC):
                        nc.vector.tensor_copy(
                            v_sb[st][:, 65 * h:65 * h + ADIM],
                            ps[:, ADIM * h:ADIM * h + ADIM])

            # ---- attention for a pair of heads (own pool scope) ----
            def run_heads(p3, heads, tail=None):
                spsum = p3.enter_context(
                    tc.tile_pool(name="spsum", bufs=2, space="PSUM"))
                cpsum = p3.enter_context(
                    tc.tile_pool(name="cpsum", bufs=2, space="PSUM"))
                epool = p3.enter_context(tc.tile_pool(name="expT", bufs=4))
                bpool = p3.enter_context(tc.tile_pool(name="band", bufs=3))
                rpool = p3.enter_context(tc.tile_pool(name="recip", bufs=2))
                gpool = p3.enter_context(tc.tile_pool(name="recB", bufs=2))
                ksrc = (kT_sb, kLo_sb, kHi_sb)
                band_tiles, ctx_tiles = {}, {}

                def issue_scores(h, th, st):
                    mt, po = h // 2, ADIM * (h % 2)
                    s0 = st * P
                    if th == 0 and st == 0:
                        bt = bpool.tile([P, NST * 3 * P], BF16, name="band")
                        nc.sync.dma_start(out=bt[:], in_=band_d[h])
                        band_tiles[h] = bt
                    band_sb = band_tiles[h]
                    expT = epool.tile([P, TH], BF16, name="expT")
                    ps = spsum.tile([P, TH], FP32, name="scores")
                    runs = []
                    for tt in range(8 * th, 8 * th + 8):
                        dd = st - tt
                        kv = 1 if dd >= 2 else (2 if dd <= -2 else 0)
                        if runs and runs[-1][2] == kv and (tt % 4) != 0:
                            runs[-1][1] = tt + 1
                        else:
                            runs.append([tt, tt + 1, kv])
                    for ta, tb, kv in runs:
                        co = (ta - 8 * th) * P
                        nc.tensor.matmul(
                            ps[:, co:co + (tb - ta) * P],
                            lhsT=ksrc[kv][mt][po:po + ADIM, s0:s0 + P],
                            rhs=qT_sb[mt][po:po + ADIM, ta * P:tb * P],
                            start=True, stop=True)
                    nc.scalar.activation(
                        expT[:], ps[:], mybir.ActivationFunctionType.Exp)
                    ta = max(st - 1, 8 * th)
                    tb = min(st + 2, 8 * th + 8)
                    if ta < tb:
                        bo = (st * 3 + (ta - st + 1)) * P
                        co = (ta - 8 * th) * P
                        w_band = (tb - ta) * P
                        nc.vector.tensor_mul(
                            expT[:, co:co + w_band],
                            expT[:, co:co + w_band],
                            band_sb[:, bo:bo + w_band])
                    return expT

                def issue_av(h, th, st, expT):
                    if st == 0:
                        ctx_tiles[(h, th)] = cpsum.tile([P, TH], FP32,
                                                        name="ctx")
                    ctx_ps = ctx_tiles[(h, th)]
                    for nb in range(2):
                        nc.tensor.matmul(
                            ctx_ps[0:65, nb * 512:nb * 512 + 512],
                            lhsT=v_sb[st][:, 65 * h:65 * h + 65],
                            rhs=expT[:, nb * 512:nb * 512 + 512],
                            start=(st == 0), stop=(st == NST - 1))
                    if st == NST - 1:
                        mt, po = h // 2, ADIM * (h % 2)
                        den = rpool.tile([1, TH], FP32, name="den")
                        rec = rpool.tile([1, TH], FP32, name="rec")
                        recB = gpool.tile([ADIM, TH], FP32, name="recB")
                        # psum row -> sbuf first: approx recip's bit trick
                        # needs IEEE fp32, not the e10m23 psum format
                        nc.vector.tensor_copy(den[:], ctx_ps[ADIM:ADIM + 1, :])
                        nc.vector.reciprocal_approx_fast(rec[:], den[:])
                        nc.gpsimd.partition_broadcast(recB[:], rec[0:1, :])
                        nc.vector.tensor_mul(
                            ctxT_sb[mt][po:po + ADIM, th * TH:th * TH + TH],
                            ctx_ps[0:ADIM, :], recB[:])

                from collections import deque
                pend = deque()
                for g in [(h, th, st) for h in heads
                          for th in range(2) for st in range(NST)]:
                    expT = issue_scores(*g)
                    pend.append((g, expT))
                    if len(pend) > 2:   # AV runs two groups behind scores
                        pg, pe = pend.popleft()
                        issue_av(*pg, pe)
                while pend:
                    pg, pe = pend.popleft()
                    issue_av(*pg, pe)
                if tail is not None:
                    for tt in range(NST):
                        tail((spsum, cpsum), tt)

            # ---- P4: out projection, inside the heads {2,3} scope,
            # drawing psum from the scores ring (no scope-exit barrier) ----
            ostage = stack.enter_context(tc.tile_pool(name="ostage", bufs=3))

            def out_proj(pools, tt):
                spsum, cpsum = pools
                st_t = ostage.tile([P, D], BF16, name="ost")
                for nb in range(2):
                    if (2 * tt + nb) % 2 == 0:
                        ps = spsum.tile([P, TH], FP32, name="scores")
                    else:
                        ps = cpsum.tile([P, TH], FP32, name="ctx")
                    for cc in range(2):
                        nc.tensor.matmul(
                            ps[:, 0:512],
                            lhsT=ctxT_sb[cc][:, tt * P:tt * P + P],
                            rhs=wo_sb[:, cc * D + nb * 512:
                                      cc * D + nb * 512 + 512],
                            start=(cc == 0), stop=(cc == 1))
                    if nb == 0:
                        nc.scalar.activation(
                            st_t[:, 0:512], ps[:, 0:512],
                            mybir.ActivationFunctionType.Copy)
                    else:
                        nc.vector.tensor_copy(
                            st_t[:, 512:1024], ps[:, 0:512])
                    nc.sync.dma_start(
                        out=out_d[tt * P:tt * P + P,
                                  nb * 512:nb * 512 + 512],
                        in_=st_t[:, nb * 512:nb * 512 + 512])

            with ExitStack() as p3:
                run_heads(p3, (0, 1, 2, 3), tail=out_proj)
    nc.compile()
    return nc


def _bf16(x):
    return np.ascontiguousarray(np.asarray(x, np.float32)).astype(
        ml_dtypes.bfloat16)


def _fp8(x):
    x = np.clip(np.ascontiguousarray(np.asarray(x, np.float32)), -240, 240)
    return x.astype(ml_dtypes.float8_e4m3fn)


def _swiz(xT):
    """[D, S]-like -> SBUF layout [128, (D/128)*S] (chunk kc at cols kc*S)."""
    d0, s0 = xT.shape
    return np.ascontiguousarray(
        xT.reshape(d0 // P, P, s0).transpose(1, 0, 2).reshape(P, -1))


def _swiz_v(xT):
    """s-major chunks: [128, ch(4), kc(8), 512] so the v projection can
    start on the first quarter of xv."""
    d0, s0 = xT.shape
    a = xT.reshape(d0 // P, P, 4, s0 // 4)          # [kc, p, ch, 512]
    return np.ascontiguousarray(
        a.transpose(1, 2, 0, 3).reshape(P, -1))


def _host_inputs(iQ, iK, iV, Wq, Wk, Wv, Wo, rel_pemb):
    iQ, iK, iV = (np.asarray(a, np.float32) for a in (iQ, iK, iV))
    Wq, Wk, Wv, Wo = (np.asarray(a, np.float32) for a in (Wq, Wk, Wv, Wo))
    rel_pemb = np.asarray(rel_pemb, np.float32)
    iQ = iQ * 0.125  # fold the 1/sqrt(ADIM) scale into the q input
    pembT = rel_pemb.T
    pemb = np.stack([np.tile(rel_pemb[0], 2), np.tile(rel_pemb[32], 2)],
                    axis=1).astype(np.float32)  # [128, 2]: col0=hi, col1=lo

    sl = np.arange(P)[:, None]
    tl = np.arange(P)[None, :]
    idx_d = {d: np.clip(d + sl - tl + K_REL, 0, NJ - 1) for d in (128, 0, -128)}
    slot_d = (128, 0, -128)

    in_maps = []
    for c in range(8):
        b, g = c // 4, c % 4
        cols = slice(DHC * g, DHC * g + DHC)
        Qg = iQ[b] @ Wq[:, cols]  # already includes the 1/8 scale
        band = np.zeros((HPC, NST, 3, P, P), np.float32)
        for h in range(HPC):
            ph = Qg[:, ADIM * h:ADIM * h + ADIM] @ pembT
            for st in range(NST):
                for slot, d in enumerate(slot_d):
                    tt = st - 1 + slot
                    if not 0 <= tt < NST:
                        continue
                    pb = ph[tt * P:tt * P + P]
                    band[h, st, slot] = pb[tl, idx_d[d]]
        band = np.exp(band)
        # -> [HPC, 128(sl), NST*3*128(tl-groups)]
        band = np.ascontiguousarray(band.transpose(0, 3, 1, 2, 4)
                                    .reshape(HPC, P, NST * 3 * P))
        in_maps.append({
            "xq": _bf16(_swiz(iQ[b].T)), "xk": _bf16(_swiz(iK[b].T)),
            "xv": _bf16(_swiz_v(iV[b].T)),
            "wq": _bf16(_swiz(Wq[:, cols])), "wk": _bf16(_swiz(Wk[:, cols])),
            "wv": _bf16(_swiz(Wv[:, cols])), "wo": _bf16(_swiz(Wo[cols, :])),
            "pemb": pemb, "band": _bf16(band),
        })
    return in_maps


def kernel(iQ, iK, iV, Wq, Wk, Wv, Wo, rel_pemb, _trace=False):
    global _COMPILED
    if _COMPILED is None:
        _COMPILED = build_nc()
    nc = _COMPILED
    in_maps = _host_inputs(iQ, iK, iV, Wq, Wk, Wv, Wo, rel_pemb)
    res = run_bass_kernel_spmd(nc, in_maps, list(range(8)), trace=_trace)
    parts = [res.results[c]["out"].astype(np.float32) for c in range(8)]
    out = np.stack([parts[0] + parts[1] + parts[2] + parts[3],
                    parts[4] + parts[5] + parts[6] + parts[7]])
    if _trace:
        return out, res
    return out
